# revision 46
# baseline (speedup 1.0000x reference)
"""Causal scaled-dot attention (single head, with Q/K/V projections) on 8
Trainium2 NeuronCores. Data-parallel over batch: core c computes batch c.

Shapes (hardcoded): B=8, S=2048, H=1024, f32 I/O.
Returns (context [B,S,H], attention_weights [B,S,S]) like the reference.
"""

import contextlib
import json

import numpy as np

import concourse.bass as bass
import concourse.mybir as mybir
import concourse.tile as tile
from concourse.bass_utils import run_bass_kernel_spmd
from concourse.masks import make_identity, make_upper_triangular

P = 128
S = 2048
H = 1024
B = 8
TB = 512               # block width (one PSUM bank of f32)
NTB = S // TB          # 4 query blocks
HC = H // P            # 8 hidden chunks
SCALE = float(1.0 / np.sqrt(np.float32(H)))

F32 = mybir.dt.float32
BF16 = mybir.dt.bfloat16


def _split_excess_waits(bir_bytes: bytes) -> bytes:
    """This walrus build accepts at most 1 sync wait per instruction (2 for
    EventSemaphore); Tile emits more. Hoist excess waits onto wait-only
    EventSemaphore instructions inserted just before, on the same engine."""
    bir = json.loads(bir_bytes)
    ctr = 0
    for fn in bir["functions"]:
        for blk in fn["blocks"]:
            out = []
            changed = False
            for inst in blk["instructions"]:
                si = inst.get("sync_info")
                waits = (si or {}).get("on_wait") or []
                cap = 2 if inst.get("opcode") == "EventSemaphore" else 1
                if len(waits) > cap:
                    excess, keep = waits[:-cap], waits[-cap:]
                    for i in range(0, len(excess), 2):
                        ctr += 1
                        out.append({
                            "debug": inst.get("debug", 0),
                            "engine": inst["engine"],
                            "ins": [], "outs": [],
                            "name": f"EVW-{ctr}-{inst['name']}",
                            "opcode": "EventSemaphore",
                            "sync_info": {"on_update": [],
                                          "on_wait": excess[i:i + 2]},
                        })
                    si["on_wait"] = keep
                    changed = True
                out.append(inst)
            if changed:
                blk["instructions"] = out
    return json.dumps(bir).encode()


def build_nc(loop_k: int = 1) -> bass.Bass:
    nc = bass.Bass("TRN2")

    queries = nc.dram_tensor("queries", [S, H], F32, kind="ExternalInput")
    keys = nc.dram_tensor("keys", [S, H], F32, kind="ExternalInput")
    values = nc.dram_tensor("values", [S, H], F32, kind="ExternalInput")
    Wq = nc.dram_tensor("Wq", [H, H], F32, kind="ExternalInput")
    Wk = nc.dram_tensor("Wk", [H, H], F32, kind="ExternalInput")
    Wv = nc.dram_tensor("Wv", [H, H], F32, kind="ExternalInput")
    bq = nc.dram_tensor("bq", [H], F32, kind="ExternalInput")
    bk = nc.dram_tensor("bk", [H], F32, kind="ExternalInput")
    bv = nc.dram_tensor("bv", [H], F32, kind="ExternalInput")
    ctx_out = nc.dram_tensor("ctx", [S, H], F32, kind="ExternalOutput")
    attn_out = nc.dram_tensor("attn", [S, S], F32, kind="ExternalOutput")

    with tile.TileContext(nc) as tc:
        if loop_k > 1:
            # Timing harness: repeat the (idempotent) body on-device so the
            # per-iteration time can be extracted from host wall-clock.
            with tc.For_i(0, loop_k, 1):
                _emit(nc, tc, queries, keys, values, Wq, Wk, Wv,
                      bq, bk, bv, ctx_out, attn_out)
        else:
            _emit(nc, tc, queries, keys, values, Wq, Wk, Wv, bq, bk, bv,
                  ctx_out, attn_out)

    orig = nc.to_json_bytes
    nc.to_json_bytes = lambda: _split_excess_waits(orig())
    return nc


def _emit(nc, tc, queries, keys, values, Wq, Wk, Wv, bq, bk, bv,
          ctx_out, attn_out):
    ctx = contextlib.ExitStack()
    with ctx:
        consts = ctx.enter_context(tc.tile_pool(name="consts", bufs=1))
        stg = ctx.enter_context(tc.tile_pool(name="stg", bufs=2))
        stgbf = ctx.enter_context(tc.tile_pool(name="stgbf", bufs=2))
        wt_pool = ctx.enter_context(tc.tile_pool(name="wt", bufs=10))
        xt_pool = ctx.enter_context(tc.tile_pool(name="xt", bufs=8))
        kt_pool = ctx.enter_context(tc.tile_pool(name="kt", bufs=8))
        qt_pool = ctx.enter_context(tc.tile_pool(name="qt", bufs=8))
        v_pool = ctx.enter_context(tc.tile_pool(name="v", bufs=16))
        e_pool = ctx.enter_context(tc.tile_pool(name="e", bufs=17))
        small = ctx.enter_context(tc.tile_pool(name="small", bufs=2))
        ostage = ctx.enter_context(tc.tile_pool(name="ostage", bufs=2))

        dram = ctx.enter_context(
            tc.tile_pool(name="dram", bufs=3, space="DRAM"))
        ps_psum = ctx.enter_context(
            tc.tile_pool(name="ps_psum", bufs=2, space="PSUM"))
        sums_psum = ctx.enter_context(
            tc.tile_pool(name="sums_psum", bufs=1, space="PSUM"))
        c_psum = ctx.enter_context(
            tc.tile_pool(name="c_psum", bufs=2, space="PSUM"))
        sc_psum = ctx.enter_context(
            tc.tile_pool(name="sc_psum", bufs=1, space="PSUM"))

        # ---- constants ----
        identity = consts.tile([P, P], BF16)
        make_identity(nc, identity)
        tri_mask = consts.tile([P, P], BF16)  # 1 where s <= t (keep), 0 below
        make_upper_triangular(nc, tri_mask, val=1.0, diag=True)
        ones_col = consts.tile([P, 1], BF16)
        nc.vector.memset(ones_col, 1.0)
        ones_row_f32 = consts.tile([1, P], F32)
        nc.vector.memset(ones_row_f32, 1.0)

        # biases: column layout [p, o] where h = o*P + p
        bq_col = consts.tile([P, HC], F32)
        nc.gpsimd.dma_start(bq_col, bq.rearrange("(o p) -> p o", p=P))
        bk_col = consts.tile([P, HC], F32)
        nc.gpsimd.dma_start(bk_col, bk.rearrange("(o p) -> p o", p=P))
        bv_f32 = stg.tile([1, H], F32, tag="stg", name="bv_f32")
        nc.gpsimd.dma_start(bv_f32, bv[None, :])
        bv_row = consts.tile([1, H], BF16)
        nc.vector.tensor_copy(bv_row, bv_f32)

        def stage_bf16(src, rgroup, xbf):
            """Load rows [rgroup*512, +512) of src f32, cast to bf16, store
            into the DRAM scratch xbf (same [R, C] layout, bf16).
            Two 128-row chunks per DMA to halve instruction count."""
            C = src.shape[1]
            src3 = src.rearrange("(n p) c -> n p c", p=P)
            xbf3 = xbf.rearrange("(n p) c -> n p c", p=P)
            for j in range(2):
                r = rgroup * 4 + 2 * j
                xf = stg.tile([P, 2, C], F32, tag="stg", name="xf")
                nc.sync.dma_start(
                    xf, src3[r:r + 2].rearrange("n p c -> p n c"))
                xb = stgbf.tile([P, 2, C], BF16, tag="stgbf", name="xb")
                nc.vector.tensor_copy(xb, xf)
                nc.sync.dma_start(
                    xbf3[r:r + 2].rearrange("n p c -> p n c"), xb)

        def load_transposed2(xbf, rgroup2):
            """DMA-transpose srcT tiles out of the bf16 scratch: returns
            {c: [P, 2*TB] tile} = srcT[c*P:(c+1)*P, rgroup2*1024:+1024]."""
            C = xbf.shape[1]
            out = {}
            for c in range(C // P):
                t = xt_pool.tile([P, 2 * TB], BF16, tag="xt", name="xtt")
                nc.sync.dma_start_transpose(
                    t, xbf[rgroup2 * 2 * TB:(rgroup2 + 1) * 2 * TB,
                           c * P:(c + 1) * P])
                out[c] = t
            return out

        def transpose_x_pe(src, rgroup):
            """PE-transpose rows [rgroup*512,+512) of src f32 into
            {hc: [P, TB]} bf16 tiles (no DRAM round trip)."""
            src3 = src.rearrange("(n p) c -> n p c", p=P)
            bfts = []
            for j in range(2):
                r = rgroup * 4 + 2 * j
                xf = stg.tile([P, 2, H], F32, tag="stg", name="xpf")
                nc.sync.dma_start(
                    xf, src3[r:r + 2].rearrange("n p c -> p n c"))
                xb = stgbf.tile([P, 2, H], BF16, tag="stgbf", name="xpb")
                nc.vector.tensor_copy(xb, xf)
                bfts.append(xb)
            out = {}
            for hc in range(HC):
                pst = ps_psum.tile([P, TB], BF16, tag="ps", name="pstx")
                for j4 in range(4):
                    nc.tensor.transpose(
                        pst[:, j4 * P:(j4 + 1) * P],
                        bfts[j4 // 2][:, j4 % 2, hc * P:(hc + 1) * P],
                        identity)
                t = xt_pool.tile([P, TB], BF16, tag="xt", name="xpt")
                if hc % 2 == 0:
                    nc.vector.tensor_copy(t, pst)
                else:
                    nc.scalar.copy(t, pst)
                out[hc] = t
            return out

        def transpose_weights_pe(Wsrc, tag):
            """WT[hc] = W.T rows [hc*P,+P) as [P, H] bf16 tiles, via PE
            transposes (fills PE idle in the DMA-bound projection phases and
            keeps the weight bytes off the DMA critical path)."""
            Wsrc3 = Wsrc.rearrange("(n p) c -> n p c", p=P)
            wt = {hc: wt_pool.tile([P, H], BF16, tag="wt", name=f"wt_{tag}")
                  for hc in range(HC)}
            for rg in range(H // TB):
                bfts = []
                for j in range(2):
                    r = rg * 4 + 2 * j
                    wf = stg.tile([P, 2, H], F32, tag="stg", name="wf")
                    nc.sync.dma_start(
                        wf, Wsrc3[r:r + 2].rearrange("n p c -> p n c"))
                    wb = stgbf.tile([P, 2, H], BF16, tag="stgbf", name="wb")
                    nc.vector.tensor_copy(wb, wf)
                    bfts.append(wb)
                for hc in range(HC):
                    pst = ps_psum.tile([P, TB], BF16, tag="ps", name="pst")
                    for j4 in range(4):
                        nc.tensor.transpose(
                            pst[:, j4 * P:(j4 + 1) * P],
                            bfts[j4 // 2][:, j4 % 2, hc * P:(hc + 1) * P],
                            identity)
                    if hc % 2 == 0:
                        nc.vector.tensor_copy(
                            wt[hc][:, rg * TB:(rg + 1) * TB], pst)
                    else:
                        nc.scalar.copy(
                            wt[hc][:, rg * TB:(rg + 1) * TB], pst)
            return wt

        Exp = mybir.ActivationFunctionType.Exp
        Ident = mybir.ActivationFunctionType.Identity

        # ---- projections: qT, kT in [h, t] layout; v0 natural [s, h] ----
        qt = {m: qt_pool.tile([P, S], BF16, tag="qt", name="qtt")
              for m in range(HC)}
        kt = {m: kt_pool.tile([P, S], BF16, tag="kt", name="ktt")
              for m in range(HC)}
        v0 = {}
        xbf_q = dram.tile([S, H], BF16, tag="xbf", name="xbf_q")
        xbf_k = dram.tile([S, H], BF16, tag="xbf", name="xbf_k")
        xbf_v = dram.tile([S, H], BF16, tag="xbf", name="xbf_v")
        attn3 = attn_out.rearrange("(n p) t -> n p t", p=P)

        def proj_qk(rg2, xbf, wt, dst, b_col):
            xt = load_transposed2(xbf, rg2)
            for sub in range(2):
                rg = 2 * rg2 + sub
                for m in range(HC):
                    psq = ps_psum.tile([P, TB], F32, tag="ps", name="psq")
                    for hc in range(HC):
                        nc.tensor.matmul(
                            psq, wt[hc][:, m * P:(m + 1) * P],
                            xt[hc][:, sub * TB:(sub + 1) * TB],
                            start=(hc == 0), stop=(hc == 7))
                    nc.scalar.activation(dst[m][:, rg * TB:(rg + 1) * TB],
                                         psq, Ident, bias=b_col[:, m:m + 1])

        def proj_v(rg2, wvt):
            xtv = load_transposed2(xbf_v, rg2)
            for dd in range(8):
                i = 8 * rg2 + dd
                psv = c_psum.tile([P, H], F32, tag="c", name="psv")
                for hc in range(HC):
                    nc.tensor.matmul(psv[:, 0:TB],
                                     xtv[hc][:, dd * P:(dd + 1) * P],
                                     wvt[hc][:, 0:TB],
                                     start=(hc == 0), stop=(hc == 7))
                    nc.tensor.matmul(psv[:, TB:H],
                                     xtv[hc][:, dd * P:(dd + 1) * P],
                                     wvt[hc][:, TB:H],
                                     start=(hc == 0), stop=(hc == 7))
                v0[i] = v_pool.tile([P, H], BF16, tag="v", name="v0t")
                nc.scalar.copy(v0[i], psv)  # no bias — folded into context

        def scores_block(tb):
            n_i = 4 * tb + 4
            psum_sums = sums_psum.tile([1, TB], F32, tag="sums", name="pss")
            E = {}
            for i in range(n_i):
                pss = ps_psum.tile([P, TB], F32, tag="ps", name="pssc")
                # columns left of the causal diagonal are never read: skip them
                d = max(0, i - 4 * tb)
                for m in range(HC):
                    nc.tensor.matmul(pss[:, d * P:TB],
                                     kt[m][:, i * P:(i + 1) * P],
                                     qt[m][:, tb * TB + d * P:(tb + 1) * TB],
                                     start=(m == 0), stop=(m == 7))
                E[i] = e_pool.tile([P, TB], BF16, tag="e", name="et")
                if i < 4 * tb:
                    nc.scalar.activation(E[i], pss, Exp, scale=SCALE)
                else:
                    if d > 0:
                        nc.gpsimd.memset(E[i][:, 0:d * P], 0.0)
                    nc.scalar.activation(E[i][:, d * P:TB],
                                         pss[:, d * P:TB], Exp, scale=SCALE)
                    nc.vector.tensor_tensor(
                        E[i][:, d * P:(d + 1) * P],
                        E[i][:, d * P:(d + 1) * P],
                        tri_mask, mybir.AluOpType.mult)
                nc.tensor.matmul(psum_sums, ones_col, E[i],
                                 start=(i == 0), stop=(i == n_i - 1),
                                 skip_group_check=True)

            # normalization pieces
            recip_row = small.tile([1, TB], F32, tag="rrow", name="rrow")
            nc.vector.reciprocal(recip_row, psum_sums)
            sums_row_bf = small.tile([1, TB], BF16, tag="srow", name="srow")
            nc.scalar.copy(sums_row_bf, psum_sums)
            psb = ps_psum.tile([P, TB], F32, tag="ps", name="psb")
            nc.tensor.matmul(psb, ones_row_f32, recip_row,
                             start=True, stop=True)
            bcast = small.tile([P, TB], F32, tag="bcast", name="bcast")
            nc.vector.tensor_copy(bcast, psb)

            # attention weights out (only causal blocks; rest stays 0);
            # two key-chunks share one staging tile and one DMA
            for g in range(n_i // 2):
                at2 = ostage.tile([P, 2, TB], F32, tag="at", name="at2")
                for j in range(2):
                    nc.vector.tensor_tensor(at2[:, j, :], E[2 * g + j],
                                            bcast, mybir.AluOpType.mult)
                nc.sync.dma_start(
                    attn3[2 * g:2 * g + 2, :,
                          tb * TB:(tb + 1) * TB].rearrange("n p t -> p n t"),
                    at2)

            return E, sums_row_bf

        def ctx_block(tb, E, sums_row_bf):
            # context for the 4 query chunks of this tb
            for d in range(4):
                j4 = 4 * tb + d
                psc = c_psum.tile([P, H], F32, tag="c", name="psc")
                pssc = sc_psum.tile([P, 1], F32, tag="sc", name="psscol")
                for i in range(j4 + 1):
                    lhs = E[i][:, d * P:(d + 1) * P]
                    nc.tensor.matmul(psc[:, 0:TB], lhs, v0[i][:, 0:TB],
                                     start=(i == 0), stop=False,
                                     skip_group_check=True)
                    nc.tensor.matmul(psc[:, TB:H], lhs, v0[i][:, TB:H],
                                     start=(i == 0), stop=False,
                                     skip_group_check=True)
                    nc.tensor.matmul(pssc, lhs, ones_col,
                                     start=(i == 0), stop=(i == j4),
                                     skip_group_check=True)
                # fold v-bias: ctx += sums[t] * bv  (softmax cols sum to 1)
                nc.tensor.matmul(psc[:, 0:TB],
                                 sums_row_bf[0:1, d * P:(d + 1) * P],
                                 bv_row[0:1, 0:TB],
                                 start=False, stop=True,
                                 skip_group_check=True)
                nc.tensor.matmul(psc[:, TB:H],
                                 sums_row_bf[0:1, d * P:(d + 1) * P],
                                 bv_row[0:1, TB:H],
                                 start=False, stop=True,
                                 skip_group_check=True)
                recip_col = small.tile([P, 1], F32, tag="rcol", name="rcol")
                nc.vector.reciprocal(recip_col, pssc)
                ct = ostage.tile([P, H], F32, tag="ct", name="ct")
                nc.vector.tensor_scalar_mul(ct, psc, recip_col)
                nc.sync.dma_start(ctx_out[j4 * P:(j4 + 1) * P, :], ct)

        # interleave DMA-heavy staging/projection with PE-heavy attention:
        # after half the sequence is projected, query blocks 0-1 are ready.
        wqt = transpose_weights_pe(Wq, "wq")
        for rg in range(NTB):
            xtq = transpose_x_pe(queries, rg)
            for m in range(HC):
                psq = ps_psum.tile([P, TB], F32, tag="ps", name="psq")
                for hc in range(HC):
                    nc.tensor.matmul(
                        psq, wqt[hc][:, m * P:(m + 1) * P], xtq[hc],
                        start=(hc == 0), stop=(hc == 7))
                nc.scalar.activation(qt[m][:, rg * TB:(rg + 1) * TB],
                                     psq, Ident, bias=bq_col[:, m:m + 1])
            # prefetch k staging into this phase's PE window
            stage_bf16(keys, rg, xbf_k)
        wkt = transpose_weights_pe(Wk, "wk")
        for rg2 in range(NTB // 2):
            proj_qk(rg2, xbf_k, wkt, kt, bk_col)
            stage_bf16(values, 2 * rg2, xbf_v)
            stage_bf16(values, 2 * rg2 + 1, xbf_v)
        wvt = transpose_weights_pe(Wv, "wv")
        for rg2 in range(NTB // 2):
            proj_v(rg2, wvt)
        for tb in range(NTB):
            ctx_block(tb, *scores_block(tb))


_CACHED = {}


def _get_nc():
    if "nc" not in _CACHED:
        _CACHED["nc"] = build_nc()
    return _CACHED["nc"]


def kernel(queries, keys, values, Wq, bq, Wk, bk, Wv, bv):
    nc = _get_nc()
    f32 = np.float32
    in_maps = []
    for c in range(B):
        in_maps.append({
            "queries": np.ascontiguousarray(queries[c], dtype=f32),
            "keys": np.ascontiguousarray(keys[c], dtype=f32),
            "values": np.ascontiguousarray(values[c], dtype=f32),
            "Wq": np.ascontiguousarray(Wq, dtype=f32),
            "Wk": np.ascontiguousarray(Wk, dtype=f32),
            "Wv": np.ascontiguousarray(Wv, dtype=f32),
            "bq": np.ascontiguousarray(bq, dtype=f32),
            "bk": np.ascontiguousarray(bk, dtype=f32),
            "bv": np.ascontiguousarray(bv, dtype=f32),
        })
    res = run_bass_kernel_spmd(nc, in_maps, core_ids=list(range(B)))
    context = np.stack([res.results[c]["ctx"] for c in range(B)])
    attn = np.stack([res.results[c]["attn"] for c in range(B)])
    return context, attn


# revision 47
# speedup vs baseline: 1.0051x; 1.0051x over previous
"""Causal scaled-dot attention (single head, with Q/K/V projections) on 8
Trainium2 NeuronCores. Data-parallel over batch: core c computes batch c.

Shapes (hardcoded): B=8, S=2048, H=1024, f32 I/O.
Returns (context [B,S,H], attention_weights [B,S,S]) like the reference.
"""

import contextlib
import json

import numpy as np

import concourse.bass as bass
import concourse.mybir as mybir
import concourse.tile as tile
from concourse.bass_utils import run_bass_kernel_spmd
from concourse.masks import make_identity, make_upper_triangular

P = 128
S = 2048
H = 1024
B = 8
TB = 512               # block width (one PSUM bank of f32)
NTB = S // TB          # 4 query blocks
HC = H // P            # 8 hidden chunks
SCALE = float(1.0 / np.sqrt(np.float32(H)))

F32 = mybir.dt.float32
BF16 = mybir.dt.bfloat16


def _split_excess_waits(bir_bytes: bytes) -> bytes:
    """This walrus build accepts at most 1 sync wait per instruction (2 for
    EventSemaphore); Tile emits more. Hoist excess waits onto wait-only
    EventSemaphore instructions inserted just before, on the same engine."""
    bir = json.loads(bir_bytes)
    ctr = 0
    for fn in bir["functions"]:
        for blk in fn["blocks"]:
            out = []
            changed = False
            for inst in blk["instructions"]:
                si = inst.get("sync_info")
                waits = (si or {}).get("on_wait") or []
                cap = 2 if inst.get("opcode") == "EventSemaphore" else 1
                if len(waits) > cap:
                    excess, keep = waits[:-cap], waits[-cap:]
                    for i in range(0, len(excess), 2):
                        ctr += 1
                        out.append({
                            "debug": inst.get("debug", 0),
                            "engine": inst["engine"],
                            "ins": [], "outs": [],
                            "name": f"EVW-{ctr}-{inst['name']}",
                            "opcode": "EventSemaphore",
                            "sync_info": {"on_update": [],
                                          "on_wait": excess[i:i + 2]},
                        })
                    si["on_wait"] = keep
                    changed = True
                out.append(inst)
            if changed:
                blk["instructions"] = out
    return json.dumps(bir).encode()


def build_nc(loop_k: int = 1) -> bass.Bass:
    nc = bass.Bass("TRN2")

    queries = nc.dram_tensor("queries", [S, H], F32, kind="ExternalInput")
    keys = nc.dram_tensor("keys", [S, H], F32, kind="ExternalInput")
    values = nc.dram_tensor("values", [S, H], F32, kind="ExternalInput")
    Wq = nc.dram_tensor("Wq", [H, H], F32, kind="ExternalInput")
    Wk = nc.dram_tensor("Wk", [H, H], F32, kind="ExternalInput")
    Wv = nc.dram_tensor("Wv", [H, H], F32, kind="ExternalInput")
    bq = nc.dram_tensor("bq", [H], F32, kind="ExternalInput")
    bk = nc.dram_tensor("bk", [H], F32, kind="ExternalInput")
    bv = nc.dram_tensor("bv", [H], F32, kind="ExternalInput")
    ctx_out = nc.dram_tensor("ctx", [S, H], F32, kind="ExternalOutput")
    attn_out = nc.dram_tensor("attn", [S, S], F32, kind="ExternalOutput")

    with tile.TileContext(nc) as tc:
        if loop_k > 1:
            # Timing harness: repeat the (idempotent) body on-device so the
            # per-iteration time can be extracted from host wall-clock.
            with tc.For_i(0, loop_k, 1):
                _emit(nc, tc, queries, keys, values, Wq, Wk, Wv,
                      bq, bk, bv, ctx_out, attn_out)
        else:
            _emit(nc, tc, queries, keys, values, Wq, Wk, Wv, bq, bk, bv,
                  ctx_out, attn_out)

    orig = nc.to_json_bytes
    nc.to_json_bytes = lambda: _split_excess_waits(orig())
    return nc


def _emit(nc, tc, queries, keys, values, Wq, Wk, Wv, bq, bk, bv,
          ctx_out, attn_out):
    ctx = contextlib.ExitStack()
    with ctx:
        consts = ctx.enter_context(tc.tile_pool(name="consts", bufs=1))
        stg = ctx.enter_context(tc.tile_pool(name="stg", bufs=2))
        stgbf = ctx.enter_context(tc.tile_pool(name="stgbf", bufs=2))
        wt_pool = ctx.enter_context(tc.tile_pool(name="wt", bufs=10))
        xt_pool = ctx.enter_context(tc.tile_pool(name="xt", bufs=8))
        kt_pool = ctx.enter_context(tc.tile_pool(name="kt", bufs=8))
        qt_pool = ctx.enter_context(tc.tile_pool(name="qt", bufs=8))
        v_pool = ctx.enter_context(tc.tile_pool(name="v", bufs=16))
        e_pool = ctx.enter_context(tc.tile_pool(name="e", bufs=17))
        small = ctx.enter_context(tc.tile_pool(name="small", bufs=2))
        ostage = ctx.enter_context(tc.tile_pool(name="ostage", bufs=2))

        dram = ctx.enter_context(
            tc.tile_pool(name="dram", bufs=3, space="DRAM"))
        ps_psum = ctx.enter_context(
            tc.tile_pool(name="ps_psum", bufs=2, space="PSUM"))
        sums_psum = ctx.enter_context(
            tc.tile_pool(name="sums_psum", bufs=1, space="PSUM"))
        c_psum = ctx.enter_context(
            tc.tile_pool(name="c_psum", bufs=2, space="PSUM"))
        sc_psum = ctx.enter_context(
            tc.tile_pool(name="sc_psum", bufs=1, space="PSUM"))

        # ---- constants ----
        identity = consts.tile([P, P], BF16)
        make_identity(nc, identity)
        tri_mask = consts.tile([P, P], BF16)  # 1 where s <= t (keep), 0 below
        make_upper_triangular(nc, tri_mask, val=1.0, diag=True)
        ones_col = consts.tile([P, 1], BF16)
        nc.vector.memset(ones_col, 1.0)
        ones_row_bf = consts.tile([1, P], BF16)
        nc.vector.memset(ones_row_bf, 1.0)
        one_11 = consts.tile([1, 1], F32)
        nc.vector.memset(one_11, 1.0)

        # biases: column layout [p, o] where h = o*P + p
        bq_col = consts.tile([P, HC], F32)
        nc.gpsimd.dma_start(bq_col, bq.rearrange("(o p) -> p o", p=P))
        bk_col = consts.tile([P, HC], F32)
        nc.gpsimd.dma_start(bk_col, bk.rearrange("(o p) -> p o", p=P))
        bv_f32 = stg.tile([1, H], F32, tag="stg", name="bv_f32")
        nc.gpsimd.dma_start(bv_f32, bv[None, :])
        bv_row = consts.tile([1, H], BF16)
        nc.vector.tensor_copy(bv_row, bv_f32)

        def stage_bf16(src, rgroup, xbf):
            """Load rows [rgroup*512, +512) of src f32, cast to bf16, store
            into the DRAM scratch xbf (same [R, C] layout, bf16).
            Two 128-row chunks per DMA to halve instruction count."""
            C = src.shape[1]
            src3 = src.rearrange("(n p) c -> n p c", p=P)
            xbf3 = xbf.rearrange("(n p) c -> n p c", p=P)
            for j in range(2):
                r = rgroup * 4 + 2 * j
                xf = stg.tile([P, 2, C], F32, tag="stg", name="xf")
                nc.sync.dma_start(
                    xf, src3[r:r + 2].rearrange("n p c -> p n c"))
                xb = stgbf.tile([P, 2, C], BF16, tag="stgbf", name="xb")
                nc.vector.tensor_copy(xb, xf)
                nc.sync.dma_start(
                    xbf3[r:r + 2].rearrange("n p c -> p n c"), xb)

        def load_transposed2(xbf, rgroup2):
            """DMA-transpose srcT tiles out of the bf16 scratch: returns
            {c: [P, 2*TB] tile} = srcT[c*P:(c+1)*P, rgroup2*1024:+1024]."""
            C = xbf.shape[1]
            out = {}
            for c in range(C // P):
                t = xt_pool.tile([P, 2 * TB], BF16, tag="xt", name="xtt")
                nc.sync.dma_start_transpose(
                    t, xbf[rgroup2 * 2 * TB:(rgroup2 + 1) * 2 * TB,
                           c * P:(c + 1) * P])
                out[c] = t
            return out

        def transpose_x_pe(src, rgroup):
            """PE-transpose rows [rgroup*512,+512) of src f32 into
            {hc: [P, TB]} bf16 tiles (no DRAM round trip)."""
            src3 = src.rearrange("(n p) c -> n p c", p=P)
            bfts = []
            for j in range(2):
                r = rgroup * 4 + 2 * j
                xf = stg.tile([P, 2, H], F32, tag="stg", name="xpf")
                nc.sync.dma_start(
                    xf, src3[r:r + 2].rearrange("n p c -> p n c"))
                xb = stgbf.tile([P, 2, H], BF16, tag="stgbf", name="xpb")
                nc.vector.tensor_copy(xb, xf)
                bfts.append(xb)
            out = {}
            for hc in range(HC):
                pst = ps_psum.tile([P, TB], BF16, tag="ps", name="pstx")
                for j4 in range(4):
                    nc.tensor.transpose(
                        pst[:, j4 * P:(j4 + 1) * P],
                        bfts[j4 // 2][:, j4 % 2, hc * P:(hc + 1) * P],
                        identity)
                t = xt_pool.tile([P, TB], BF16, tag="xt", name="xpt")
                if hc % 2 == 0:
                    nc.vector.tensor_copy(t, pst)
                else:
                    nc.scalar.copy(t, pst)
                out[hc] = t
            return out

        def transpose_weights_pe(Wsrc, tag):
            """WT[hc] = W.T rows [hc*P,+P) as [P, H] bf16 tiles, via PE
            transposes (fills PE idle in the DMA-bound projection phases and
            keeps the weight bytes off the DMA critical path)."""
            Wsrc3 = Wsrc.rearrange("(n p) c -> n p c", p=P)
            wt = {hc: wt_pool.tile([P, H], BF16, tag="wt", name=f"wt_{tag}")
                  for hc in range(HC)}
            for rg in range(H // TB):
                bfts = []
                for j in range(2):
                    r = rg * 4 + 2 * j
                    wf = stg.tile([P, 2, H], F32, tag="stg", name="wf")
                    nc.sync.dma_start(
                        wf, Wsrc3[r:r + 2].rearrange("n p c -> p n c"))
                    wb = stgbf.tile([P, 2, H], BF16, tag="stgbf", name="wb")
                    nc.vector.tensor_copy(wb, wf)
                    bfts.append(wb)
                for hc in range(HC):
                    pst = ps_psum.tile([P, TB], BF16, tag="ps", name="pst")
                    for j4 in range(4):
                        nc.tensor.transpose(
                            pst[:, j4 * P:(j4 + 1) * P],
                            bfts[j4 // 2][:, j4 % 2, hc * P:(hc + 1) * P],
                            identity)
                    if hc % 2 == 0:
                        nc.vector.tensor_copy(
                            wt[hc][:, rg * TB:(rg + 1) * TB], pst)
                    else:
                        nc.scalar.copy(
                            wt[hc][:, rg * TB:(rg + 1) * TB], pst)
            return wt

        Exp = mybir.ActivationFunctionType.Exp
        Ident = mybir.ActivationFunctionType.Identity

        # ---- projections: qT, kT in [h, t] layout; v0 natural [s, h] ----
        qt = {m: qt_pool.tile([P, S], BF16, tag="qt", name="qtt")
              for m in range(HC)}
        kt = {m: kt_pool.tile([P, S], BF16, tag="kt", name="ktt")
              for m in range(HC)}
        v0 = {}
        xbf_q = dram.tile([S, H], BF16, tag="xbf", name="xbf_q")
        xbf_k = dram.tile([S, H], BF16, tag="xbf", name="xbf_k")
        xbf_v = dram.tile([S, H], BF16, tag="xbf", name="xbf_v")
        attn3 = attn_out.rearrange("(n p) t -> n p t", p=P)

        def proj_qk(rg2, xbf, wt, dst, b_col):
            xt = load_transposed2(xbf, rg2)
            for sub in range(2):
                rg = 2 * rg2 + sub
                for m in range(HC):
                    psq = ps_psum.tile([P, TB], F32, tag="ps", name="psq")
                    for hc in range(HC):
                        nc.tensor.matmul(
                            psq, wt[hc][:, m * P:(m + 1) * P],
                            xt[hc][:, sub * TB:(sub + 1) * TB],
                            start=(hc == 0), stop=(hc == 7))
                    nc.scalar.activation(dst[m][:, rg * TB:(rg + 1) * TB],
                                         psq, Ident, bias=b_col[:, m:m + 1])

        def proj_v(rg2, wvt):
            xtv = load_transposed2(xbf_v, rg2)
            for dd in range(8):
                i = 8 * rg2 + dd
                psv = c_psum.tile([P, H], F32, tag="c", name="psv")
                for hc in range(HC):
                    nc.tensor.matmul(psv[:, 0:TB],
                                     xtv[hc][:, dd * P:(dd + 1) * P],
                                     wvt[hc][:, 0:TB],
                                     start=(hc == 0), stop=(hc == 7))
                    nc.tensor.matmul(psv[:, TB:H],
                                     xtv[hc][:, dd * P:(dd + 1) * P],
                                     wvt[hc][:, TB:H],
                                     start=(hc == 0), stop=(hc == 7))
                v0[i] = v_pool.tile([P, H], BF16, tag="v", name="v0t")
                nc.scalar.copy(v0[i], psv)  # no bias — folded into context

        def scores_block(tb):
            n_i = 4 * tb + 4
            psum_sums = sums_psum.tile([1, TB], F32, tag="sums", name="pss")
            E = {}
            for i in range(n_i):
                pss = ps_psum.tile([P, TB], F32, tag="ps", name="pssc")
                # columns left of the causal diagonal are never read: skip them
                d = max(0, i - 4 * tb)
                for m in range(HC):
                    nc.tensor.matmul(pss[:, d * P:TB],
                                     kt[m][:, i * P:(i + 1) * P],
                                     qt[m][:, tb * TB + d * P:(tb + 1) * TB],
                                     start=(m == 0), stop=(m == 7))
                E[i] = e_pool.tile([P, TB], BF16, tag="e", name="et")
                if i < 4 * tb:
                    nc.scalar.activation(E[i], pss, Exp, scale=SCALE)
                else:
                    if d > 0:
                        nc.gpsimd.memset(E[i][:, 0:d * P], 0.0)
                    nc.scalar.activation(E[i][:, d * P:TB],
                                         pss[:, d * P:TB], Exp, scale=SCALE)
                    nc.vector.tensor_tensor(
                        E[i][:, d * P:(d + 1) * P],
                        E[i][:, d * P:(d + 1) * P],
                        tri_mask, mybir.AluOpType.mult)
                nc.tensor.matmul(psum_sums, ones_col, E[i],
                                 start=(i == 0), stop=(i == n_i - 1),
                                 skip_group_check=True)

            # normalization pieces
            recip_row = small.tile([1, TB], F32, tag="rrow", name="rrow")
            nc.vector.reciprocal(recip_row, psum_sums)
            sums_row_bf = small.tile([1, TB], BF16, tag="srow", name="srow")
            nc.scalar.copy(sums_row_bf, psum_sums)
            recip_row_bf = small.tile([1, TB], BF16, tag="rrbf", name="rrbf")
            nc.scalar.copy(recip_row_bf, recip_row)
            psb = ps_psum.tile([P, TB], F32, tag="ps", name="psb")
            nc.tensor.matmul(psb, ones_row_bf, recip_row_bf,
                             start=True, stop=True)
            bcast = small.tile([P, TB], F32, tag="bcast", name="bcast")
            nc.vector.tensor_copy(bcast, psb)

            # attention weights out (only causal blocks; rest stays 0);
            # two key-chunks share one staging tile and one DMA
            for g in range(n_i // 2):
                at2 = ostage.tile([P, 2, TB], F32, tag="at", name="at2")
                for j in range(2):
                    nc.vector.tensor_tensor(at2[:, j, :], E[2 * g + j],
                                            bcast, mybir.AluOpType.mult)
                nc.sync.dma_start(
                    attn3[2 * g:2 * g + 2, :,
                          tb * TB:(tb + 1) * TB].rearrange("n p t -> p n t"),
                    at2)

            return E, sums_row_bf, recip_row

        def ctx_block(tb, E, sums_row_bf, recip_row):
            for d in range(4):
                j4 = 4 * tb + d
                psc = c_psum.tile([P, H], F32, tag="c", name="psc")
                pssc = sc_psum.tile([P, 1], F32, tag="sc", name="psscol")
                nc.tensor.transpose(
                    pssc, recip_row[0:1, d * P:(d + 1) * P], one_11)
                for i in range(j4 + 1):
                    lhs = E[i][:, d * P:(d + 1) * P]
                    nc.tensor.matmul(psc[:, 0:TB], lhs, v0[i][:, 0:TB],
                                     start=(i == 0), stop=False,
                                     skip_group_check=True)
                    nc.tensor.matmul(psc[:, TB:H], lhs, v0[i][:, TB:H],
                                     start=(i == 0), stop=False,
                                     skip_group_check=True)
                # fold v-bias: ctx += sums[t] * bv  (softmax cols sum to 1)
                nc.tensor.matmul(psc[:, 0:TB],
                                 sums_row_bf[0:1, d * P:(d + 1) * P],
                                 bv_row[0:1, 0:TB],
                                 start=False, stop=True,
                                 skip_group_check=True)
                nc.tensor.matmul(psc[:, TB:H],
                                 sums_row_bf[0:1, d * P:(d + 1) * P],
                                 bv_row[0:1, TB:H],
                                 start=False, stop=True,
                                 skip_group_check=True)
                recip_col = small.tile([P, 1], F32, tag="rcol", name="rcol")
                nc.vector.tensor_copy(recip_col, pssc)
                ct = ostage.tile([P, H], F32, tag="ct", name="ct")
                nc.vector.tensor_scalar_mul(ct, psc, recip_col)
                nc.sync.dma_start(ctx_out[j4 * P:(j4 + 1) * P, :], ct)

        # interleave DMA-heavy staging/projection with PE-heavy attention:
        # after half the sequence is projected, query blocks 0-1 are ready.
        wqt = transpose_weights_pe(Wq, "wq")
        for rg in range(NTB):
            xtq = transpose_x_pe(queries, rg)
            for m in range(HC):
                psq = ps_psum.tile([P, TB], F32, tag="ps", name="psq")
                for hc in range(HC):
                    nc.tensor.matmul(
                        psq, wqt[hc][:, m * P:(m + 1) * P], xtq[hc],
                        start=(hc == 0), stop=(hc == 7))
                nc.scalar.activation(qt[m][:, rg * TB:(rg + 1) * TB],
                                     psq, Ident, bias=bq_col[:, m:m + 1])
            # prefetch k staging into this phase's PE window
            stage_bf16(keys, rg, xbf_k)
        wkt = transpose_weights_pe(Wk, "wk")
        for rg2 in range(NTB // 2):
            proj_qk(rg2, xbf_k, wkt, kt, bk_col)
            stage_bf16(values, 2 * rg2, xbf_v)
            stage_bf16(values, 2 * rg2 + 1, xbf_v)
        wvt = transpose_weights_pe(Wv, "wv")
        for rg2 in range(NTB // 2):
            proj_v(rg2, wvt)
        for tb in range(NTB):
            ctx_block(tb, *scores_block(tb))


_CACHED = {}


def _get_nc():
    if "nc" not in _CACHED:
        _CACHED["nc"] = build_nc()
    return _CACHED["nc"]


def kernel(queries, keys, values, Wq, bq, Wk, bk, Wv, bv):
    nc = _get_nc()
    f32 = np.float32
    in_maps = []
    for c in range(B):
        in_maps.append({
            "queries": np.ascontiguousarray(queries[c], dtype=f32),
            "keys": np.ascontiguousarray(keys[c], dtype=f32),
            "values": np.ascontiguousarray(values[c], dtype=f32),
            "Wq": np.ascontiguousarray(Wq, dtype=f32),
            "Wk": np.ascontiguousarray(Wk, dtype=f32),
            "Wv": np.ascontiguousarray(Wv, dtype=f32),
            "bq": np.ascontiguousarray(bq, dtype=f32),
            "bk": np.ascontiguousarray(bk, dtype=f32),
            "bv": np.ascontiguousarray(bv, dtype=f32),
        })
    res = run_bass_kernel_spmd(nc, in_maps, core_ids=list(range(B)))
    context = np.stack([res.results[c]["ctx"] for c in range(B)])
    attn = np.stack([res.results[c]["attn"] for c in range(B)])
    return context, attn
